# revision 13
# baseline (speedup 1.0000x reference)
"""Trainium2 Bass kernel for nn_DistillLossContrastive.

Contract: kernel(**inputs) takes the FULL unsharded inputs and returns the
FULL output (a scalar f32 loss). Internally the BS axis is sharded across 8
NeuronCores (core b owns object b): each core computes its 32 pooled mask
features with a [32,8192]x[8192,512] matmul chain, the [513,32] pooled block
is all-gathered, and every core redundantly computes the global 256x256
contrastive logits matrix + masked diagonal log-softmax losses on device.
"""

import numpy as np

import concourse.bass as bass
import concourse.mybir as mybir
import concourse.tile as tile
from concourse import bacc
from concourse.bass_utils import run_bass_kernel_spmd
from concourse.masks import make_identity

BS, NPTS, NM, D = 8, 8192, 32, 512
NCORES = 8
NT = BS * NM          # 256 total masks
KT = NPTS // 128      # 64 contraction tiles of 128 points
KT_PER_DMA = 8        # 8 k-tiles per DMA -> [128, 8*512] f32 = 2 MiB
NDMA = KT // KT_PER_DMA
DC = D // 128         # 4 chunks of the feature dim

F32 = mybir.dt.float32
F32R = mybir.dt.float32r
AX_X = mybir.AxisListType.X
ALU = mybir.AluOpType
ACT = mybir.ActivationFunctionType


def _build_program():
    nc = bacc.Bacc(
        "TRN2",
        target_bir_lowering=False,
        debug=False,
        num_devices=NCORES,
    )

    net = nc.dram_tensor("net", [NPTS, D], F32, kind="ExternalInput")
    maskT = nc.dram_tensor("maskT", [128, KT * NM], F32, kind="ExternalInput")
    membT = nc.dram_tensor("membT", [D, NT], F32, kind="ExternalInput")
    lscale = nc.dram_tensor("lscale", [1, 1], F32, kind="ExternalInput")
    out = nc.dram_tensor("out", [1, 1], F32, kind="ExternalOutput")

    with tile.TileContext(nc) as tc:
        with (
            tc.tile_pool(name="consts", bufs=1) as consts,
            tc.tile_pool(name="netp", bufs=3) as netp,
            tc.tile_pool(name="maskp", bufs=1) as maskp,
            tc.tile_pool(name="work", bufs=1) as work,
            tc.tile_pool(name="ph2", bufs=1) as ph2,
            tc.tile_pool(name="ps1", bufs=1, space="PSUM") as ps1,
            tc.tile_pool(name="ps2", bufs=1, space="PSUM") as ps2,
            tc.tile_pool(name="dramp", bufs=1, space="DRAM") as dramp,
        ):
            # ---- constants / small loads (scalar HWDGE ring, overlap net stream) ----
            ones_col = consts.tile([128, 1], F32)
            nc.vector.memset(ones_col[:], 1.0)
            ones_row = consts.tile([1, 128], F32)
            nc.vector.memset(ones_row[:], 1.0)
            id128 = consts.tile([128, 128], F32)
            make_identity(nc, id128[:])
            id32 = consts.tile([32, 32], F32)
            make_identity(nc, id32[:])

            maskT_sb = maskp.tile([128, KT * NM], F32R)
            nc.scalar.dma_start(maskT_sb[:], maskT[:, :].bitcast(F32R))

            mbT = []
            for c in range(DC):
                t = ph2.tile([128, NT], F32R, name=f"mbT{c}")
                nc.scalar.dma_start(
                    t[:], membT[c * 128 : (c + 1) * 128, :].bitcast(F32R)
                )
                mbT.append(t)
            sc_sb = consts.tile([1, 1], F32)
            nc.scalar.dma_start(sc_sb[:], lscale[:, :])

            # ---- phase 1: sum_feats[b] = mask_pts[b] @ net[b]  (PSUM accumulate) ----
            ps_feats = ps1.tile([NM, D], F32)
            net_v = net.rearrange("(ko ks p) d -> ko p ks d", p=128, ks=KT_PER_DMA)
            for i in range(NDMA):
                nt = netp.tile([128, KT_PER_DMA * D], F32R, name="nt")
                nc.sync.dma_start(
                    nt[:].rearrange("p (ks d) -> p ks d", d=D), net_v[i].bitcast(F32R)
                )
                for ks in range(KT_PER_DMA):
                    k = i * KT_PER_DMA + ks
                    nc.tensor.matmul(
                        ps_feats[:, :],
                        maskT_sb[:, k * NM : (k + 1) * NM],
                        nt[:, ks * D : (ks + 1) * D],
                        start=(k == 0),
                        stop=(k == KT - 1),
                    )

            # ---- counts: cnt[m] = sum_p mask[m, p] ----
            part = work.tile([128, NM], F32)
            nc.vector.reduce_sum(
                part[:],
                maskT_sb[:].bitcast(F32).rearrange("p (k m) -> p m k", k=KT),
                axis=AX_X,
            )
            ps_cntF = ps1.tile([1, NM], F32)
            nc.tensor.matmul(ps_cntF[:, :], ones_col[:], part[:])
            cntF = work.tile([1, NM], F32)
            nc.vector.tensor_copy(cntF[:], ps_cntF[:])
            ps_cntP = ps1.tile([NM, 1], F32)
            nc.tensor.matmul(ps_cntP[:, :], part[:], ones_col[:])
            dn = work.tile([NM, 1], F32)
            nc.vector.tensor_scalar_add(dn[:], ps_cntP[:], 1e-12)
            rec = work.tile([NM, 1], F32)
            nc.vector.reciprocal(rec[:], dn[:])

            # ---- avg + transpose to [512, 32] for the all-gather ----
            avg = work.tile([NM, D], F32)
            nc.vector.tensor_scalar_mul(avg[:], ps_feats[:], rec[:])
            agin_sb = work.tile([128, DC * NM], F32)
            for c in range(DC):
                pst = ps1.tile([128, NM], F32, name="pst")
                nc.tensor.matmul(
                    pst[:, :],
                    avg[:, c * 128 : (c + 1) * 128],
                    id32[:],
                    start=True,
                    stop=True,
                )
                nc.vector.tensor_copy(agin_sb[:, c * NM : (c + 1) * NM], pst[:])

            ag_in = dramp.tile([D + 1, NM], F32)
            ag_out = dramp.tile([NCORES * (D + 1), NM], F32, addr_space="Shared")
            nc.sync.dma_start(
                ag_in[0:D, :].rearrange("(c p) m -> p c m", p=128),
                agin_sb[:].rearrange("p (c m) -> p c m", m=NM),
            )
            nc.sync.dma_start(ag_in[D : D + 1, :], cntF[:])
            nc.gpsimd.collective_compute(
                "AllGather",
                ALU.bypass,
                replica_groups=[list(range(NCORES))],
                ins=[ag_in[:].opt()],
                outs=[ag_out[:].opt()],
            )

            # ---- phase 2 (all cores redundantly): logits + losses ----
            agv = ag_out.rearrange("(b r) m -> b r m", b=NCORES)
            avT = []
            for c in range(DC):
                t = ph2.tile([128, NT], F32R, name=f"avT{c}")
                nc.sync.dma_start(
                    t[:].rearrange("p (b m) -> p b m", m=NM),
                    agv[:, c * 128 : (c + 1) * 128, :]
                    .rearrange("b p m -> p b m")
                    .bitcast(F32R),
                )
                avT.append(t)

            # broadcast exp(logit_scale) to all partitions
            ps_es = ps2.tile([128, 1], F32)
            nc.tensor.matmul(ps_es[:, :], ones_row[:], sc_sb[:])
            es = ph2.tile([128, 1], F32)
            nc.scalar.activation(es[:], ps_es[:], ACT.Exp)

            # logits chunks (lg[r]: rows 128r..128r+127 of logits) and transposed
            lg = {}
            for which, lhs_set, rhs_set in (("L", mbT, avT), ("T", avT, mbT)):
                for r in range(2):
                    ps_lg = ps2.tile(
                        [128, NT], F32, name=f"pslg{which}{r}", tag="pslg", bufs=2
                    )
                    for c in range(DC):
                        nc.tensor.matmul(
                            ps_lg[:, :],
                            lhs_set[c][:, r * 128 : (r + 1) * 128],
                            rhs_set[c][:],
                            start=(c == 0),
                            stop=(c == DC - 1),
                        )
                    t = ph2.tile([128, NT], F32, name=f"lg{which}{r}")
                    nc.scalar.activation(t[:], ps_lg[:], ACT.Copy, scale=es[:])
                    lg[(which, r)] = t

            # per-row logsumexp of both orientations + diagonal + validity
            ps_fin = ps2.tile([1, 4], F32)
            for r in range(2):
                lse = {}
                for which in ("L", "T"):
                    t = lg[(which, r)]
                    mx = ph2.tile([128, 1], F32, name=f"mx{which}{r}")
                    nc.vector.reduce_max(mx[:], t[:], axis=AX_X, negate=True)
                    ex = ph2.tile([128, NT], F32, name="ex", tag="ex", bufs=2)
                    sm = ph2.tile([128, 1], F32, name=f"sm{which}{r}")
                    nc.scalar.activation(
                        ex[:], t[:], ACT.Exp, bias=mx[:], accum_out=sm[:]
                    )
                    ln_t = ph2.tile([128, 1], F32, name=f"ln{which}{r}")
                    nc.scalar.activation(ln_t[:], sm[:], ACT.Ln)
                    l_t = ph2.tile([128, 1], F32, name=f"lse{which}{r}")
                    nc.vector.tensor_sub(l_t[:], ln_t[:], mx[:])
                    lse[which] = l_t

                dg = ph2.tile([128, 1], F32, name=f"dg{r}")
                dsc = ph2.tile([128, 128], F32, name="dsc", tag="dsc", bufs=2)
                nc.vector.tensor_mul(
                    dsc[:], lg[("L", r)][:, r * 128 : (r + 1) * 128], id128[:]
                )
                nc.vector.reduce_sum(dg[:], dsc[:], axis=AX_X)

                vc = ph2.tile([128, 1], F32, name=f"vc{r}")
                nc.sync.dma_start(vc[:], agv[4 * r : 4 * r + 4, D, :])
                gt = ph2.tile([128, 1], F32, name=f"gt{r}")
                nc.vector.tensor_scalar(gt[:], vc[:], 0.0, None, op0=ALU.is_gt)

                stk = ph2.tile([128, 4], F32, name=f"stk{r}")
                for j, which in ((0, "L"), (2, "T")):
                    tmp = ph2.tile([128, 1], F32, name=f"tmp{which}{r}")
                    nc.vector.tensor_sub(tmp[:], lse[which][:], dg[:])
                    nc.vector.tensor_mul(stk[:, j : j + 1], tmp[:], gt[:])
                    nc.vector.tensor_scalar(
                        stk[:, j + 1 : j + 2], stk[:, j : j + 1], 0.0, None,
                        op0=ALU.is_gt,
                    )
                nc.tensor.matmul(
                    ps_fin[:, :], ones_col[:], stk[:], start=(r == 0), stop=(r == 1)
                )

            # ---- final scalar: mean over positive losses, both directions ----
            fin = ph2.tile([1, 4], F32, name="fin")
            nc.vector.tensor_copy(fin[:], ps_fin[:])
            res = ph2.tile([1, 5], F32, name="res")
            # res columns: denomL, recipL, valL, valT, result
            for j, (s_col, c_col) in enumerate(((0, 1), (2, 3))):
                nc.vector.tensor_scalar_max(
                    res[:, 0:1], fin[:, c_col : c_col + 1], 1.0
                )
                nc.vector.reciprocal(res[:, 1:2], res[:, 0:1])
                nc.vector.tensor_mul(
                    res[:, 2 + j : 3 + j], fin[:, s_col : s_col + 1], res[:, 1:2]
                )
                gtc = ph2.tile([1, 1], F32, name=f"gtc{j}")
                nc.vector.tensor_scalar(
                    gtc[:], fin[:, c_col : c_col + 1], 0.0, None, op0=ALU.is_gt
                )
                nc.vector.tensor_mul(
                    res[:, 2 + j : 3 + j], res[:, 2 + j : 3 + j], gtc[:]
                )
            nc.vector.tensor_add(res[:, 4:5], res[:, 2:3], res[:, 3:4])
            out_sb = ph2.tile([1, 1], F32, name="out_sb")
            nc.vector.tensor_scalar_mul(out_sb[:], res[:, 4:5], 0.5)
            nc.sync.dma_start(out[:, :], out_sb[:])

    nc.compile()
    return nc


_NC_CACHE = None


def _get_program():
    global _NC_CACHE
    if _NC_CACHE is None:
        _NC_CACHE = _build_program()
    return _NC_CACHE


def _make_in_maps(net_out, mask_embs, mask_pts, logit_scale):
    net_out = np.asarray(net_out, dtype=np.float32)
    mask_embs = np.asarray(mask_embs, dtype=np.float32)
    mask_pts = np.asarray(mask_pts, dtype=np.float32)
    membT = np.ascontiguousarray(mask_embs.T)
    ls = np.array(logit_scale, dtype=np.float32).reshape(1, 1)
    in_maps = []
    for b in range(NCORES):
        net_b = np.ascontiguousarray(net_out[b * NPTS : (b + 1) * NPTS])
        m = mask_pts[b]  # [NM, NPTS]
        maskT_b = np.ascontiguousarray(
            m.reshape(NM, KT, 128).transpose(2, 1, 0).reshape(128, KT * NM)
        )
        in_maps.append(
            {"net": net_b, "maskT": maskT_b, "membT": membT, "lscale": ls}
        )
    return in_maps


def _install_ntff_shim():
    """Provide the antenv.axon_hooks registry this image lacks and register
    the ctypes NTFF hook (same as trn_agent_boot would)."""
    import sys
    import types

    if "antenv.axon_hooks" not in sys.modules:
        import antenv

        mod = types.ModuleType("antenv.axon_hooks")
        holder = [None]
        mod.set_axon_ntff_profile_hook = lambda h: holder.__setitem__(0, h)
        mod.get_axon_ntff_profile_hook = lambda: holder[0]
        sys.modules["antenv.axon_hooks"] = mod
        antenv.axon_hooks = mod
    from antenv.axon_hooks import (
        get_axon_ntff_profile_hook,
        set_axon_ntff_profile_hook,
    )

    if get_axon_ntff_profile_hook() is None:
        from trn_agent_boot.trn_boot import _ntff_profile_via_ctypes

        set_axon_ntff_profile_hook(
            _ntff_profile_via_ctypes("/opt/axon/libaxon_pjrt.so")
        )


def run(net_out, mask_embs, mask_pts, logit_scale, pt_offset=None, trace=False):
    """Run on 8 NeuronCores; returns (scalar ndarray, BassKernelResults)."""
    if trace:
        try:
            _install_ntff_shim()
        except Exception:
            pass
    nc = _get_program()
    in_maps = _make_in_maps(net_out, mask_embs, mask_pts, logit_scale)
    r = run_bass_kernel_spmd(nc, in_maps, core_ids=list(range(NCORES)), trace=trace)
    val = np.asarray(r.results[0]["out"], dtype=np.float32).reshape(())
    return val, r


def kernel(net_out, mask_embs, mask_pts, logit_scale, pt_offset=None):
    val, _ = run(net_out, mask_embs, mask_pts, logit_scale, pt_offset)
    return val


# revision 15
# speedup vs baseline: 1.2511x; 1.2511x over previous
"""Trainium2 Bass kernel for nn_DistillLossContrastive.

Contract: kernel(**inputs) takes the FULL unsharded inputs and returns the
FULL output (a scalar f32 loss). Internally the BS axis is sharded across 8
NeuronCores (core b owns object b): each core computes its 32 pooled mask
features with a [32,8192]x[8192,512] matmul chain, the [513,32] pooled block
is all-gathered, and every core redundantly computes the global 256x256
contrastive logits matrix + masked diagonal log-softmax losses on device.
"""

import numpy as np

import concourse.bass as bass
import concourse.mybir as mybir
import concourse.tile as tile
from concourse import bacc
from concourse.bass_utils import run_bass_kernel_spmd
from concourse.masks import make_identity

BS, NPTS, NM, D = 8, 8192, 32, 512
NCORES = 8
NT = BS * NM          # 256 total masks
KT = NPTS // 128      # 64 contraction tiles of 128 points
KT_PER_DMA = 8        # 8 k-tiles per DMA -> [128, 8*512] f32 = 2 MiB
NDMA = KT // KT_PER_DMA
DC = D // 128         # 4 chunks of the feature dim

F32 = mybir.dt.float32
F32R = mybir.dt.float32r
AX_X = mybir.AxisListType.X
ALU = mybir.AluOpType
ACT = mybir.ActivationFunctionType


def _build_program():
    nc = bacc.Bacc(
        "TRN2",
        target_bir_lowering=False,
        debug=False,
        num_devices=NCORES,
    )

    net = nc.dram_tensor("net", [NPTS, D], F32, kind="ExternalInput")
    maskT = nc.dram_tensor("maskT", [128, KT * NM], F32, kind="ExternalInput")
    membT = nc.dram_tensor("membT", [D, NT], F32, kind="ExternalInput")
    lscale = nc.dram_tensor("lscale", [1, 1], F32, kind="ExternalInput")
    out = nc.dram_tensor("out", [1, 1], F32, kind="ExternalOutput")

    with tile.TileContext(nc) as tc:
        with (
            tc.tile_pool(name="consts", bufs=1) as consts,
            tc.tile_pool(name="netp", bufs=4) as netp,
            tc.tile_pool(name="maskp", bufs=1) as maskp,
            tc.tile_pool(name="work", bufs=1) as work,
            tc.tile_pool(name="ph2", bufs=1) as ph2,
            tc.tile_pool(name="ps", bufs=1, space="PSUM") as ps,
            tc.tile_pool(name="dramp", bufs=1, space="DRAM") as dramp,
        ):
            # ---- inputs for phase 1 (scalar HWDGE ring; net stream on sync) ----
            maskT_sb = maskp.tile([128, KT * NM], F32R)
            nc.scalar.dma_start(maskT_sb[:], maskT[:, :].bitcast(F32R))

            # ---- phase 1: sum_feats[b] = mask_pts[b] @ net[b]  (PSUM accumulate) ----
            ps_feats = ps.tile([NM, D], F32, name="ps_feats", tag="feats")
            net_v = net.rearrange("(ko ks p) d -> ko p ks d", p=128, ks=KT_PER_DMA)
            for i in range(NDMA):
                nt = netp.tile([128, KT_PER_DMA * D], F32R, name="nt")
                nc.sync.dma_start(
                    nt[:].rearrange("p (ks d) -> p ks d", d=D), net_v[i].bitcast(F32R)
                )
                for ks in range(KT_PER_DMA):
                    k = i * KT_PER_DMA + ks
                    nc.tensor.matmul(
                        ps_feats[:, :],
                        maskT_sb[:, k * NM : (k + 1) * NM],
                        nt[:, ks * D : (ks + 1) * D],
                        start=(k == 0),
                        stop=(k == KT - 1),
                    )

            # ---- small loads + constants (low priority, overlap net stream) ----
            mbT = []
            for c in range(DC):
                t = ph2.tile([128, NT], F32R, name=f"mbT{c}")
                nc.scalar.dma_start(
                    t[:], membT[c * 128 : (c + 1) * 128, :].bitcast(F32R)
                )
                mbT.append(t)
            sc_sb = consts.tile([1, 1], F32)
            nc.scalar.dma_start(sc_sb[:], lscale[:, :])
            ones_col = consts.tile([128, 1], F32)
            nc.vector.memset(ones_col[:], 1.0)
            ones_row = consts.tile([1, 128], F32)
            nc.vector.memset(ones_row[:], 1.0)
            id128 = consts.tile([128, 128], F32)
            make_identity(nc, id128[:])
            id32 = consts.tile([32, 32], F32)
            make_identity(nc, id32[:])

            # broadcast logit_scale to all partitions, exp it (used in phase 1)
            ps_es = ps.tile([128, 1], F32, name="ps_es", tag="small", bufs=2)
            nc.tensor.matmul(ps_es[:, :], ones_row[:], sc_sb[:])
            es = consts.tile([128, 1], F32)
            nc.scalar.activation(es[:], ps_es[:], ACT.Exp)

            # ---- counts: cnt[m] = sum_p mask[m, p] (overlaps net stream) ----
            part = work.tile([128, NM], F32)
            nc.vector.reduce_sum(
                part[:],
                maskT_sb[:].bitcast(F32).rearrange("p (k m) -> p m k", k=KT),
                axis=AX_X,
            )
            ps_cntF = ps.tile([1, NM], F32, name="ps_cntF", tag="small", bufs=2)
            nc.tensor.matmul(ps_cntF[:, :], ones_col[:], part[:])
            cntF = work.tile([1, NM], F32)
            nc.vector.tensor_copy(cntF[:], ps_cntF[:])
            ps_cntP = ps.tile([NM, 1], F32, name="ps_cntP", tag="small", bufs=2)
            nc.tensor.matmul(ps_cntP[:, :], part[:], ones_col[:])
            dn = work.tile([NM, 1], F32)
            nc.vector.tensor_scalar_add(dn[:], ps_cntP[:], 1e-12)
            rec = work.tile([NM, 1], F32)
            nc.vector.reciprocal(rec[:], dn[:])
            # fold exp(logit_scale) into the per-mask reciprocal: avg rows come
            # out pre-scaled, so logits need no further scaling.
            rec2 = work.tile([NM, 1], F32)
            nc.vector.tensor_mul(rec2[:], rec[:], es[0:NM, :])

            # ---- avg*es + transpose to [512, 32] for the all-gather ----
            avg = work.tile([NM, D], F32)
            nc.vector.tensor_scalar_mul(avg[:], ps_feats[:], rec2[:])
            agin_sb = work.tile([128, DC * NM], F32)
            for c in range(DC):
                pst = ps.tile([128, NM], F32, name="pst", tag="small", bufs=2)
                nc.tensor.matmul(
                    pst[:, :],
                    avg[:, c * 128 : (c + 1) * 128],
                    id32[:],
                    start=True,
                    stop=True,
                )
                nc.vector.tensor_copy(agin_sb[:, c * NM : (c + 1) * NM], pst[:])

            ag_in = dramp.tile([D + 1, NM], F32)
            ag_out = dramp.tile([NCORES * (D + 1), NM], F32, addr_space="Shared")
            nc.sync.dma_start(
                ag_in[0:D, :].rearrange("(c p) m -> p c m", p=128),
                agin_sb[:].rearrange("p (c m) -> p c m", m=NM),
            )
            nc.sync.dma_start(ag_in[D : D + 1, :], cntF[:])
            nc.gpsimd.collective_compute(
                "AllGather",
                ALU.bypass,
                replica_groups=[list(range(NCORES))],
                ins=[ag_in[:].opt()],
                outs=[ag_out[:].opt()],
            )

            # ---- phase 2 (all cores redundantly): logits + losses ----
            agv = ag_out.rearrange("(b r) m -> b r m", b=NCORES)
            avT = []
            for c in range(DC):
                t = ph2.tile([128, NT], F32R, name=f"avT{c}")
                nc.sync.dma_start(
                    t[:].rearrange("p (b m) -> p b m", m=NM),
                    agv[:, c * 128 : (c + 1) * 128, :]
                    .rearrange("b p m -> p b m")
                    .bitcast(F32R),
                )
                avT.append(t)
            vc = {}
            for r in range(2):
                v = ph2.tile([128, 1], F32, name=f"vc{r}")
                nc.sync.dma_start(v[:], agv[4 * r : 4 * r + 4, D, :])
                vc[r] = v

            # logits chunks in PSUM (already scaled by exp(logit_scale))
            lg = {}
            for which, lhs_set, rhs_set in (("L", mbT, avT), ("T", avT, mbT)):
                for r in range(2):
                    ps_lg = ps.tile(
                        [128, NT], F32, name=f"pslg{which}{r}", tag="pslg", bufs=4
                    )
                    for c in range(DC):
                        nc.tensor.matmul(
                            ps_lg[:, :],
                            lhs_set[c][:, r * 128 : (r + 1) * 128],
                            rhs_set[c][:],
                            start=(c == 0),
                            stop=(c == DC - 1),
                        )
                    lg[(which, r)] = ps_lg

            # row-max (negated) for all four tiles, then batched Exp, then Ln

            mx = {}
            for key, t in lg.items():
                m = ph2.tile([128, 1], F32, name=f"mx{key[0]}{key[1]}")
                nc.vector.reduce_max(m[:], t[:], axis=AX_X, negate=True)
                mx[key] = m
            sm = {}
            for key, t in lg.items():
                ex = ph2.tile([128, NT], F32, name="ex", tag="ex", bufs=4)
                s = ph2.tile([128, 1], F32, name=f"sm{key[0]}{key[1]}")
                nc.scalar.activation(
                    ex[:], t[:], ACT.Exp, bias=mx[key][:], accum_out=s[:]
                )
                sm[key] = s
            lse = {}
            for key in lg:
                ln_t = ph2.tile([128, 1], F32, name=f"ln{key[0]}{key[1]}")
                nc.scalar.activation(ln_t[:], sm[key][:], ACT.Ln)
                l_t = ph2.tile([128, 1], F32, name=f"lse{key[0]}{key[1]}")
                nc.vector.tensor_sub(l_t[:], ln_t[:], mx[key][:])
                lse[key] = l_t

            ps_fin = ps.tile([1, 4], F32, name="ps_fin", tag="small", bufs=2)
            for r in range(2):
                dg = ph2.tile([128, 1], F32, name=f"dg{r}")
                dsc = ph2.tile([128, 128], F32, name="dsc", tag="dsc", bufs=2)
                nc.vector.tensor_mul(
                    dsc[:], lg[("L", r)][:, r * 128 : (r + 1) * 128], id128[:]
                )
                nc.vector.reduce_sum(dg[:], dsc[:], axis=AX_X)

                gt = ph2.tile([128, 1], F32, name=f"gt{r}")
                nc.vector.tensor_scalar(gt[:], vc[r][:], 0.0, None, op0=ALU.is_gt)

                stk = ph2.tile([128, 4], F32, name=f"stk{r}")
                for j, which in ((0, "L"), (2, "T")):
                    tmp = ph2.tile([128, 1], F32, name=f"tmp{which}{r}")
                    nc.vector.tensor_sub(tmp[:], lse[(which, r)][:], dg[:])
                    nc.vector.tensor_mul(stk[:, j : j + 1], tmp[:], gt[:])
                    nc.vector.tensor_scalar(
                        stk[:, j + 1 : j + 2], stk[:, j : j + 1], 0.0, None,
                        op0=ALU.is_gt,
                    )
                nc.tensor.matmul(
                    ps_fin[:, :], ones_col[:], stk[:], start=(r == 0), stop=(r == 1)
                )

            # ---- final scalar: mean over positive losses, both directions ----
            fin = ph2.tile([1, 4], F32, name="fin")
            nc.vector.tensor_copy(fin[:], ps_fin[:])
            res = ph2.tile([1, 5], F32, name="res")
            # res columns: denom, recip, valL, valT, result
            for j, (s_col, c_col) in enumerate(((0, 1), (2, 3))):
                nc.vector.tensor_scalar_max(
                    res[:, 0:1], fin[:, c_col : c_col + 1], 1.0
                )
                nc.vector.reciprocal(res[:, 1:2], res[:, 0:1])
                nc.vector.tensor_mul(
                    res[:, 2 + j : 3 + j], fin[:, s_col : s_col + 1], res[:, 1:2]
                )
                gtc = ph2.tile([1, 1], F32, name=f"gtc{j}")
                nc.vector.tensor_scalar(
                    gtc[:], fin[:, c_col : c_col + 1], 0.0, None, op0=ALU.is_gt
                )
                nc.vector.tensor_mul(
                    res[:, 2 + j : 3 + j], res[:, 2 + j : 3 + j], gtc[:]
                )
            nc.vector.tensor_add(res[:, 4:5], res[:, 2:3], res[:, 3:4])
            out_sb = ph2.tile([1, 1], F32, name="out_sb")
            nc.vector.tensor_scalar_mul(out_sb[:], res[:, 4:5], 0.5)
            nc.sync.dma_start(out[:, :], out_sb[:])

    nc.compile()
    return nc


_NC_CACHE = None


def _get_program():
    global _NC_CACHE
    if _NC_CACHE is None:
        _NC_CACHE = _build_program()
    return _NC_CACHE


def _make_in_maps(net_out, mask_embs, mask_pts, logit_scale):
    net_out = np.asarray(net_out, dtype=np.float32)
    mask_embs = np.asarray(mask_embs, dtype=np.float32)
    mask_pts = np.asarray(mask_pts, dtype=np.float32)
    membT = np.ascontiguousarray(mask_embs.T)
    ls = np.array(logit_scale, dtype=np.float32).reshape(1, 1)
    in_maps = []
    for b in range(NCORES):
        net_b = np.ascontiguousarray(net_out[b * NPTS : (b + 1) * NPTS])
        m = mask_pts[b]  # [NM, NPTS]
        maskT_b = np.ascontiguousarray(
            m.reshape(NM, KT, 128).transpose(2, 1, 0).reshape(128, KT * NM)
        )
        in_maps.append(
            {"net": net_b, "maskT": maskT_b, "membT": membT, "lscale": ls}
        )
    return in_maps


def _install_ntff_shim():
    """Provide the antenv.axon_hooks registry this image lacks and register
    the ctypes NTFF hook (same as trn_agent_boot would)."""
    import sys
    import types

    if "antenv.axon_hooks" not in sys.modules:
        import antenv

        mod = types.ModuleType("antenv.axon_hooks")
        holder = [None]
        mod.set_axon_ntff_profile_hook = lambda h: holder.__setitem__(0, h)
        mod.get_axon_ntff_profile_hook = lambda: holder[0]
        sys.modules["antenv.axon_hooks"] = mod
        antenv.axon_hooks = mod
    from antenv.axon_hooks import (
        get_axon_ntff_profile_hook,
        set_axon_ntff_profile_hook,
    )

    if get_axon_ntff_profile_hook() is None:
        from trn_agent_boot.trn_boot import _ntff_profile_via_ctypes

        set_axon_ntff_profile_hook(
            _ntff_profile_via_ctypes("/opt/axon/libaxon_pjrt.so")
        )


def run(net_out, mask_embs, mask_pts, logit_scale, pt_offset=None, trace=False):
    """Run on 8 NeuronCores; returns (scalar ndarray, BassKernelResults)."""
    if trace:
        try:
            _install_ntff_shim()
        except Exception:
            pass
    nc = _get_program()
    in_maps = _make_in_maps(net_out, mask_embs, mask_pts, logit_scale)
    r = run_bass_kernel_spmd(nc, in_maps, core_ids=list(range(NCORES)), trace=trace)
    val = np.asarray(r.results[0]["out"], dtype=np.float32).reshape(())
    return val, r


def kernel(net_out, mask_embs, mask_pts, logit_scale, pt_offset=None):
    val, _ = run(net_out, mask_embs, mask_pts, logit_scale, pt_offset)
    return val
